# revision 2
# baseline (speedup 1.0000x reference)
"""Entity-knowledge embedding lookup for Trainium2 (8 NeuronCores).

Math: y[t] = (1/128) * sum_{j<8} rowsum16(fact[l_tj]) @ conv_w.T + conv_b.
The 16-subvector sum and the 1x1 conv commute with the per-token label
sum, so precompute per vocab row r:
    C[r] = rowsum16(fact[r]) @ (conv_w.T / 128)        [vocab, 100] bf16
and then y[t] = sum_j C[l_tj] + b — 8 gathers of 256B per token instead
of 8 x 19.2KB, cutting per-core HBM traffic ~2x vs data-parallel gathers.

Sharding:
 - Phase 1 (vocab-sharded): the globally-referenced fact rows (np.unique
   of the 32768 labels, ~16.1k of 20k) are split contiguously across the
   8 cores; each core streams its ~2015-row shard once (~39MB sequential)
   and reduces 16 subvectors on DVE, transposes on PE, matmuls into its
   bf16 C shard. PSUM evacuations run on the scalar (Act) engine so DVE
   does only the big reduce.
 - Phase 2: AllGather of C in 4 chunks, each issued as soon as its quarter
   of the shard is done, overlapping the collective with the stream.
 - Phase 3 (token-data-parallel): per (token-group, label) indirect row
   gathers from the allgathered C, strided 8-way DVE reduce, bias, store.

The per-core tile count is derived from the actual input inside kernel()
(the program is built per call), so any label distribution is correct.
"""

import sys

import numpy as np

sys.path.insert(0, "/opt/trn_rl_repo")

import concourse.bacc as bacc
import concourse.bass as bass
import concourse.mybir as mybir
from concourse.masks import make_identity
from concourse.tile import TileContext

VOCAB = 20000
TOPK = 8
GLOVE = 300
OUTC = 100
B, L, NL = 32, 128, 8
NCORES = 8
TOKENS = B * L            # 4096
TPC = TOKENS // NCORES    # 512 tokens per core
GROUP = 128
NGROUPS = TPC // GROUP    # 4
ROW = 2 * TOPK * GLOVE    # 4800 floats per fact row
CW = 128                  # C row padded to 128 cols (bf16 -> 256B rows)
NCH = 4                   # allgather chunks

F32 = mybir.dt.float32
BF16 = mybir.dt.bfloat16
I32 = mybir.dt.int32


def build_kernel(loops=1, ntiles=16):
    """ntiles: per-core fact-shard tiles of 128 rows; must be % NCH == 0."""
    assert ntiles % NCH == 0
    vpad = ntiles * 128
    chr_ = vpad // NCH        # rows per chunk per core
    tpch = ntiles // NCH      # tiles per chunk
    cv = NCORES * vpad

    nc = bacc.Bacc(
        "TRN2", target_bir_lowering=False, debug=False, num_devices=NCORES
    )

    factp = nc.dram_tensor("factp", [vpad, ROW], F32, kind="ExternalInput").ap()
    labels = nc.dram_tensor("labels", [TPC, NL], I32, kind="ExternalInput").ap()
    wb = nc.dram_tensor("wb", [GLOVE, OUTC], F32, kind="ExternalInput").ap()
    biasrep = nc.dram_tensor("biasrep", [128, OUTC], F32, kind="ExternalInput").ap()
    y = nc.dram_tensor("y", [TPC, OUTC], F32, kind="ExternalOutput").ap()

    with TileContext(nc) as tc:
        with (
            tc.tile_pool(name="const", bufs=1) as cpool,
            tc.tile_pool(name="fact", bufs=5) as fpool,
            tc.tile_pool(name="s", bufs=4) as spool,
            tc.tile_pool(name="st", bufs=3) as stpool,
            tc.tile_pool(name="csb", bufs=3) as cbpool,
            tc.tile_pool(name="idx", bufs=NGROUPS + 1) as ipool,
            tc.tile_pool(name="g", bufs=2) as gpool,
            tc.tile_pool(name="ps_t", bufs=6, space="PSUM") as tpsum,
            tc.tile_pool(name="ps_c", bufs=2, space="PSUM") as cpsum,
            tc.tile_pool(name="dram", bufs=2, space="DRAM") as dpool,
        ):
            ident0 = cpool.tile([128, 128], F32, tag="ident0")
            make_identity(nc, ident0[:])
            ident = cpool.tile([128, 128], F32, tag="ident")
            nc.vector.tensor_copy(ident[:], ident0[:])
            wts = []
            for k in range(3):
                t0 = cpool.tile([100, OUTC], F32, tag=f"wb{k}raw")
                nc.sync.dma_start(out=t0[:], in_=wb[k * 100 : (k + 1) * 100, :])
                t = cpool.tile([100, OUTC], F32, tag=f"wb{k}")
                nc.scalar.copy(t[:], t0[:])
                wts.append(t)
            brep = cpool.tile([128, OUTC], F32, tag="brep")
            nc.sync.dma_start(out=brep[:], in_=biasrep[:])

            for _ in range(loops):
                Cshard = dpool.tile([vpad, CW], BF16, tag="Cshard")
                cf_tile = dpool.tile([cv, CW], BF16, tag="Cfull")
                Cfull = cf_tile[:]

                idxs = []
                for g in range(NGROUPS):
                    idx = ipool.tile([GROUP, NL], I32, tag="idx")
                    nc.sync.dma_start(
                        out=idx[:], in_=labels[g * GROUP : (g + 1) * GROUP, :]
                    )
                    idxs.append(idx)

                # phase 1: stream fact shard, reduce 4800 -> 300, conv to C
                for t in range(ntiles):
                    f = fpool.tile([128, ROW], F32, tag="f")
                    eng = nc.sync if t % 2 == 0 else nc.scalar
                    eng.dma_start(out=f[:], in_=factp[t * 128 : (t + 1) * 128, :])
                    s = spool.tile([128, GLOVE], F32, tag="s")
                    nc.vector.tensor_reduce(
                        out=s[:],
                        in_=f[:].rearrange("p (k c) -> p c k", k=2 * TOPK),
                        axis=mybir.AxisListType.X,
                        op=mybir.AluOpType.add,
                    )
                    sts = []
                    for k in range(3):
                        tp = tpsum.tile([100, 128], F32, tag="tp")
                        nc.tensor.transpose(
                            out=tp[:],
                            in_=s[:, k * 100 : (k + 1) * 100],
                            identity=ident[:],
                        )
                        st = stpool.tile([100, 128], F32, tag=f"st{k}")
                        nc.scalar.copy(st[:], tp[:])
                        sts.append(st)
                    cp = cpsum.tile([128, OUTC], F32, tag="cp")
                    for k in range(3):
                        nc.tensor.matmul(
                            cp[:], sts[k][:], wts[k][:],
                            start=(k == 0), stop=(k == 2),
                        )
                    csb = cbpool.tile([128, CW], BF16, tag="csb")
                    nc.scalar.copy(csb[:, :OUTC], cp[:])
                    nc.scalar.memzero(csb[:, OUTC:])
                    nc.sync.dma_start(
                        out=Cshard[t * 128 : (t + 1) * 128, :], in_=csb[:]
                    )
                    # phase 2: allgather each quarter as soon as it is done
                    if t % tpch == tpch - 1:
                        q = t // tpch
                        nc.gpsimd.collective_compute(
                            "AllGather",
                            mybir.AluOpType.bypass,
                            replica_groups=[list(range(NCORES))],
                            ins=[Cshard[q * chr_ : (q + 1) * chr_, :].opt()],
                            outs=[
                                Cfull[
                                    q * NCORES * chr_ : (q + 1) * NCORES * chr_, :
                                ].opt()
                            ],
                        )

                # phase 3: gather 8 C rows per token, reduce, bias, store
                for g in range(NGROUPS):
                    acc = gpool.tile([GROUP, NL, CW], BF16, tag="acc")
                    for j in range(NL):
                        nc.gpsimd.indirect_dma_start(
                            out=acc[:, j, :],
                            out_offset=None,
                            in_=Cfull[:],
                            in_offset=bass.IndirectOffsetOnAxis(
                                ap=idxs[g][:, j : j + 1], axis=0
                            ),
                            compute_op=mybir.AluOpType.bypass,
                        )
                    yt = spool.tile([GROUP, OUTC], F32, tag="yt")
                    nc.vector.tensor_reduce(
                        out=yt[:],
                        in_=acc[:, :, :OUTC].rearrange("p j e -> p e j"),
                        axis=mybir.AxisListType.X,
                        op=mybir.AluOpType.add,
                    )
                    yo = spool.tile([GROUP, OUTC], F32, tag="yo")
                    nc.vector.tensor_add(yo[:], yt[:], brep[:])
                    nc.sync.dma_start(
                        out=y[g * GROUP : (g + 1) * GROUP, :], in_=yo[:]
                    )

    nc.finalize()
    return nc


def plan(detect_labels):
    """Dedup plan: returns (ntiles, uniq, labr) for this input."""
    lab = np.asarray(detect_labels).reshape(TOKENS, NL).astype(np.int64)
    uniq = np.unique(lab)
    nu = len(uniq)
    s = -(-nu // NCORES)                           # rows per core (last short)
    ntiles = max(NCH, -(-s // (128 * NCH)) * NCH)  # pad to NCH-tile multiple
    vpad = ntiles * 128
    chr_ = vpad // NCH
    u = np.searchsorted(uniq, lab)                 # position in uniq
    c_of = u // s
    pos = u - c_of * s
    labr = (pos // chr_) * (NCORES * chr_) + c_of * chr_ + (pos % chr_)
    return ntiles, uniq, labr


def make_in_maps(detect_labels, fact_table, conv_w, conv_b):
    ntiles, uniq, labr = plan(detect_labels)
    vpad = ntiles * 128
    nu = len(uniq)
    s = -(-nu // NCORES)
    fact2d = np.asarray(fact_table).reshape(VOCAB, ROW).astype(np.float32)
    wbh = np.ascontiguousarray(np.asarray(conv_w).T.astype(np.float32) / 128.0)
    brep = np.ascontiguousarray(
        np.broadcast_to(
            np.asarray(conv_b).astype(np.float32)[None, :], (128, OUTC)
        )
    )
    in_maps = []
    for c in range(NCORES):
        rows = uniq[c * s : min((c + 1) * s, nu)]
        fp = np.zeros((vpad, ROW), np.float32)
        fp[: len(rows)] = fact2d[rows]
        in_maps.append(
            {
                "factp": fp,
                "labels": np.ascontiguousarray(
                    labr[c * TPC : (c + 1) * TPC].astype(np.int32)
                ),
                "wb": wbh,
                "biasrep": brep,
            }
        )
    return in_maps


def assemble_output(results):
    parts = [np.asarray(r["y"]) for r in results]  # each [512, 100]
    return np.concatenate(parts, axis=0).reshape(B, L, OUTC).astype(np.float32)


def kernel(detect_labels, fact_table, conv_w, conv_b):
    from concourse import bass_utils

    ntiles, _, _ = plan(detect_labels)
    nc = build_kernel(ntiles=ntiles)
    in_maps = make_in_maps(detect_labels, fact_table, conv_w, conv_b)
    res = bass_utils.run_bass_kernel_spmd(nc, in_maps, list(range(NCORES)))
    return assemble_output(res.results)
